# revision 1
# baseline (speedup 1.0000x reference)
"""Trainium2 Bass kernel for BilinearInteractionV2.

out[b, p, e] = (sum_d femb[b, left[p], d] * W[p, d, e]) * femb[b, right[p], e]

feature_emb: [2048, 40, 64] f32, bilinear_W: [780, 64, 64] f32,
left/right idx = upper-triangle pairs in combinations order (left-major).

Sharding: batch split across 8 NeuronCores (pure data parallel), W replicated.

Per-core plan (B_local = 256 = 2 b-tiles of 128 rows):
  - femb b-tile resident in SBUF as [128 (b), 40*64]; per-field PE
    transposes put the contraction dim d on partitions (femT [64, f*128+b])
    for the matmul stationary operand.
  - pairs grouped by left field l (combinations order is left-major, right
    fields contiguous); per group one stationary lhsT = femT field slice
    serves chunks of <=8 pairs; each chunk is one fp32r matmul
    [64,128]^T @ [64, cnt*64] -> PSUM [128, cnt*64]. Two chunks of a group
    share one PSUM tile so the DVE multiply runs at FD up to 1024.
  - DVE multiplies PSUM by the right-field slice of the resident femb tile
    into a staging tile; staging flushed to DRAM in ~3MB contiguous DMAs.
  - W is loaded once per supergroup (one DMA) and reused by both b-tiles.
    The host pre-swizzles W to [d, p, e] so each partition's DMA line is one
    long contiguous run (full HBM bandwidth) instead of 256B strided chunks.
"""

import os
import time

import numpy as np

import concourse.bass as bass
import concourse.mybir as mybir
import concourse.tile as tile
from concourse import bacc
from concourse.bass_utils import run_bass_kernel_spmd
from concourse.masks import make_identity

N_CORES = 8
BATCH = 2048
B_LOCAL = BATCH // N_CORES  # 256
B_TILES = B_LOCAL // 128    # 2
NF = 40
D = 64
PAIRS = NF * (NF - 1) // 2  # 780
F32 = mybir.dt.float32
F32R = mybir.dt.float32r

CHUNK_PAIRS = 8          # pairs per matmul: N = 512 cols = one PSUM bank
MUL_CHUNKS = int(os.environ.get("K_MUL_CHUNKS", "2"))
SG_TARGET_PAIRS = int(os.environ.get("K_SG", "64"))
STAGE_BUFS = int(os.environ.get("K_STAGE_BUFS", "4"))
W_BUFS = int(os.environ.get("K_W_BUFS", "3"))
MM_BUFS = int(os.environ.get("K_MM_BUFS", "3"))
MERGE_FLUSH = int(os.environ.get("K_MERGE_FLUSH", "0"))
FEMT_POOL = int(os.environ.get("K_FEMT_POOL", "0"))


def _mul_units():
    """Units of <=MUL_CHUNKS*CHUNK_PAIRS pairs within one left-field group.

    Yields (left_field, pair_start, [chunk_cnts], right_field_start)."""
    p0 = 0
    for l in range(NF - 1):
        k = NF - 1 - l
        for u0 in range(0, k, MUL_CHUNKS * CHUNK_PAIRS):
            npair = min(MUL_CHUNKS * CHUNK_PAIRS, k - u0)
            cnts = []
            left = npair
            while left > 0:
                cnts.append(min(CHUNK_PAIRS, left))
                left -= cnts[-1]
            yield (l, p0 + u0, cnts, l + 1 + u0)
        p0 += k


def _sg_size_plan():
    """Supergroup pair-count targets: small head (fast pipeline fill) and
    small tail (fast drain), big middle (DMA efficiency)."""
    head = [32, 64]
    tail = [48, 32, 16]
    mid = PAIRS - sum(head) - sum(tail)
    n_mid = max(1, round(mid / SG_TARGET_PAIRS + 0.5))
    base = mid // n_mid
    plan = head + [base + (1 if i < mid - base * n_mid else 0)
                   for i in range(n_mid)] + tail
    assert sum(plan) == PAIRS
    return plan


def _supergroups():
    """Group consecutive mul-units into flush units per the size plan.

    Each supergroup closes once it REACHES its target (so it may overshoot
    by up to one unit); this avoids a cascade of tiny spill groups."""
    plan = _sg_size_plan()
    sgs, cur, cnt, pi = [], [], 0, 0
    for u in _mul_units():
        cur.append(u)
        cnt += sum(u[2])
        if cnt >= plan[min(pi, len(plan) - 1)]:
            sgs.append(cur)
            cur, cnt = [], 0
            pi += 1
    if cur:
        sgs.append(cur)
    return sgs


def _sg_max_pairs():
    return max(sum(sum(u[2]) for u in sg) for sg in _supergroups())


def _body(nc, tc, pools, femb, w, out):
    femb_pool, femT_pool, ident_pool, w_pool, psT_pool, psum_pool, stage_pool = pools

    ident = ident_pool.tile([128, 128], F32)
    make_identity(nc, ident)

    # Per-b-tile resident tiles: femb rows + transposed fields. The field
    # transposes are emitted just-in-time (per supergroup) so the first
    # flush is not gated on all 80 of them.
    femb_t = []
    femT = []
    for bt in range(B_TILES):
        ft = femb_pool.tile([128, NF * D], F32, tag="femb")
        nc.sync.dma_start(
            out=ft,
            in_=femb[bt * 128:(bt + 1) * 128].rearrange("b f d -> b (f d)"),
        )
        femb_t.append(ft)
        if not FEMT_POOL:
            tt = femT_pool.tile([64, NF * 128], F32R, tag="femT")
            femT.append(tt)
    transposed = [dict() for _ in range(B_TILES)]

    def ensure_fields(bt, fields):
        for f in fields:
            if f in transposed[bt]:
                continue
            ps = psT_pool.tile([64, 128], F32)
            nc.tensor.transpose(ps, femb_t[bt][:, f * D:(f + 1) * D], ident)
            if FEMT_POOL:
                ftile = femT_pool.tile([64, 128], F32R, tag="femT")
                nc.scalar.copy(ftile, ps)
                transposed[bt][f] = ftile
            else:
                nc.scalar.copy(femT[bt][:, f * 128:(f + 1) * 128], ps)
                transposed[bt][f] = True

    def lhsT_for(bt, f):
        if FEMT_POOL:
            return transposed[bt][f]
        return femT[bt][:, f * 128:(f + 1) * 128]

    for sg in _supergroups():
        sg_p0 = sg[0][1]
        sg_np = sum(sum(u[2]) for u in sg)
        # One DMA loads this supergroup's W; both b-tiles reuse it.
        wsg = w_pool.tile([64, _sg_max_pairs(), D], F32R, tag="w")
        nc.scalar.dma_start(
            out=wsg[:, :sg_np, :],
            in_=w[:, sg_p0:sg_p0 + sg_np, :].bitcast(F32R),
        )
        if MERGE_FLUSH:
            mstage = stage_pool.tile([128, B_TILES, _sg_max_pairs() * D], F32,
                                     tag="stage")
        for bt in range(B_TILES):
            ensure_fields(bt, sorted({u[0] for u in sg}))
            if MERGE_FLUSH:
                stage = mstage[:, bt, :]
            else:
                stage = stage_pool.tile([128, _sg_max_pairs() * D], F32,
                                        tag="stage")
            for (l, p0, cnts, r0) in sg:
                un = sum(cnts)
                ps = psum_pool.tile([128, MUL_CHUNKS * CHUNK_PAIRS * D], F32,
                                    tag="mm")
                off = 0
                for cnt in cnts:
                    woff = p0 - sg_p0 + off
                    nc.tensor.matmul(
                        ps[:, off * D:(off + cnt) * D],
                        lhsT=lhsT_for(bt, l),
                        rhs=wsg[:, woff:woff + cnt, :].rearrange("d p e -> d (p e)"),
                        start=True,
                        stop=True,
                    )
                    off += cnt
                co = (p0 - sg_p0) * D
                nc.vector.tensor_mul(
                    stage[:, co:co + un * D],
                    ps[:, :un * D],
                    femb_t[bt][:, r0 * D:(r0 + un) * D],
                )
            if not MERGE_FLUSH:
                nc.sync.dma_start(
                    out=out[bt * 128:(bt + 1) * 128, sg_p0:sg_p0 + sg_np].rearrange(
                        "b p e -> b (p e)"
                    ),
                    in_=stage[:, :sg_np * D],
                )
        if MERGE_FLUSH:
            nc.sync.dma_start(
                out=out[:, sg_p0:sg_p0 + sg_np].rearrange(
                    "(t b) p e -> b t (p e)", b=128
                ),
                in_=mstage[:, :, :sg_np * D],
            )


def build_kernel(reps: int = 1) -> bass.Bass:
    """Build + finalize the per-core Bass module.

    reps > 1 wraps the whole body in a hardware loop (for timing runs)."""
    nc = bacc.Bacc("TRN2", target_bir_lowering=False)
    femb = nc.dram_tensor("feature_emb", [B_LOCAL, NF, D], F32, kind="ExternalInput")
    # host-swizzled to [d, p, e] for contiguous per-partition DMA lines
    w = nc.dram_tensor("bilinear_W", [D, PAIRS, D], F32, kind="ExternalInput")
    out = nc.dram_tensor("out", [B_LOCAL, PAIRS, D], F32, kind="ExternalOutput")

    with tile.TileContext(nc) as tc:
        with (
            tc.tile_pool(name="femb", bufs=B_TILES) as femb_pool,
            tc.tile_pool(name="femT", bufs=(12 if FEMT_POOL else B_TILES)) as femT_pool,
            tc.tile_pool(name="ident", bufs=1) as ident_pool,
            tc.tile_pool(name="w", bufs=W_BUFS) as w_pool,
            tc.tile_pool(name="psT", bufs=2, space="PSUM") as psT_pool,
            tc.tile_pool(name="mm", bufs=MM_BUFS, space="PSUM") as psum_pool,
            tc.tile_pool(name="stage", bufs=STAGE_BUFS) as stage_pool,
        ):
            pools = (femb_pool, femT_pool, ident_pool, w_pool, psT_pool,
                     psum_pool, stage_pool)
            if reps == 1:
                _body(nc, tc, pools, femb, w, out)
            else:
                with tc.For_i(0, reps, 1):
                    _body(nc, tc, pools, femb, w, out)
    nc.finalize()
    return nc


_CACHED_NC = None


def make_in_maps(feature_emb: np.ndarray, bilinear_W: np.ndarray):
    feature_emb = np.ascontiguousarray(np.asarray(feature_emb, dtype=np.float32))
    bilinear_W = np.asarray(bilinear_W, dtype=np.float32)
    assert feature_emb.shape == (BATCH, NF, D)
    assert bilinear_W.shape == (PAIRS, D, D)
    w_swz = np.ascontiguousarray(bilinear_W.transpose(1, 0, 2))  # [d, p, e]
    return [
        {
            "feature_emb": feature_emb[c * B_LOCAL:(c + 1) * B_LOCAL],
            "bilinear_W": w_swz,
        }
        for c in range(N_CORES)
    ]


def kernel(feature_emb: np.ndarray, bilinear_W: np.ndarray,
           left_idx: np.ndarray = None, right_idx: np.ndarray = None,
           **_ignored) -> np.ndarray:
    global _CACHED_NC
    if _CACHED_NC is None:
        _CACHED_NC = build_kernel(reps=1)
    nc = _CACHED_NC

    in_maps = make_in_maps(feature_emb, bilinear_W)
    # The NRT occasionally reports a transient "exec unit unrecoverable" if a
    # previous process wedged a core; it clears on retry.
    last_err = None
    for attempt in range(3):
        try:
            res = run_bass_kernel_spmd(nc, in_maps, list(range(N_CORES)))
            break
        except Exception as e:  # noqa: BLE001
            last_err = e
            time.sleep(5.0)
    else:
        raise last_err
    return np.concatenate([res.results[c]["out"] for c in range(N_CORES)], axis=0)

